# revision 1
# baseline (speedup 1.0000x reference)
"""Trainium2 Bass kernel: BFP (block-floating-point) activation quantization.

Reference semantics (input NCHW [32, 256, 56, 56] f32):
  per (batch, pixel), channels grouped in blocks of 32:
    maxabs = max |x| over the block
    e      = floor(log2(maxabs))          (guard zero blocks)
    s      = 2^(e-4)                      (5-bit mantissa, QMAX = 31)
    out    = clip(round_half_even(x / s), -31, 31) * s    (0 if maxabs == 0)

Implementation (bit-exact in fp32, validated against the reference):
  s0 = 2^e is extracted by masking the exponent bits of maxabs.  The whole
  round+clip+rescale collapses into one fused DVE op using magic-number
  rounding in the C = 1.5*2^23 * s domain:
      C  = s0 * 786432.0        (= 1.5*2^23 * 2^-4 * s0 = magic * s)
      m  = s0 * 1.9375          (= 31 * s)
      out = min(max(x + C, C - m), C + m) - C
  Every step is exact in fp32: the x + C addition performs the
  round-half-even at ULP = s, the clip bounds and the final subtraction are
  exact multiples of s in the same binade.  The outputs are +-q * 2^(e-4)
  with q <= 31 (5 significant bits), so they are exactly representable in
  bf16 — the kernel stores bf16 to HBM (half the store traffic, still
  bit-exact after the host widens to f32).

Layout: channels live on SBUF partitions after the natural NCHW DMA, but the
block reduction needs channels along the free dim, so tiles are transposed
through the (otherwise idle) tensor engine in 128x128 chunks, processed in
the pixel-on-partition layout, and transposed back in bf16.  The emission is
software-pipelined (forward transposes run two tiles ahead) so the in-order
PE queue never head-of-line blocks on a tile's backward transposes, and
DMAs are split per tile with loads on the SP queue and stores on the ACT
queue so they overlap compute instead of bracketing it.  The first tile's
load is split 4-way and its reduce split per 256 columns so the DVE starts
~2.5us earlier; the final two tails are emitted before the last forward
transposes so the closing backward/copy/store chain overlaps the last
quantizes.

The kernel is DVE-bound: the block-max reduce and the fused quantize are
both capped at 1 element/lane/cycle (the only uop the DVE has for
tensor_reduce and custom ops), giving ~63.5us of unavoidable Vector busy
per core; DMA (19.3 MB at ~358 GB/s), PE (~46us) and ACT (~29us) all sit
below it.  Profiled engine span is ~84us, identical to the measured
wall floor.

Sharding: batch 32 -> 4 per core across 8 NeuronCores; no cross-core comms.
"""

import numpy as np

import concourse.bass as bass
import concourse.mybir as mybir
from concourse import bacc, masks, tile
from concourse.bass_utils import run_bass_kernel_spmd

F32 = mybir.dt.float32
BF16 = mybir.dt.bfloat16
I32 = mybir.dt.int32

# ---------------------------------------------------------------------------
# Custom DVE op: the entire quantize in one 1x pass.
#   out = min(max(Src0 + Src1*C0, Src1*C0 - Src1*C1), Src1*C0 + Src1*C1) - Src1*C0
#   Src0 = x (pixel-major tile), Src1 = s0 = 2^e broadcast over the 32-chunk,
#   C0 = 786432.0, C1 = 1.9375.
# ---------------------------------------------------------------------------
_OP_NAME = "BFP_Q5_ANT"


def _bfp_q5_reference(in0, in1, s0, s1, imm2):
    in0 = np.asarray(in0, np.float32)
    in1 = np.asarray(in1, np.float32).reshape(in0.shape)
    c = (in1 * np.float32(s0)).astype(np.float32)
    m = (in1 * np.float32(s1)).astype(np.float32)
    u = (in0 + c).astype(np.float32)
    v = np.minimum(np.maximum(u, (c - m).astype(np.float32)),
                   (c + m).astype(np.float32)).astype(np.float32)
    return (v - c).astype(np.float32)


def _register_custom_op():
    import concourse.dve_ops as dve_ops
    from concourse.dve_ops import DveOp
    from concourse.dve_spec import C0, C1, Spec, Src0, Src1, lower, maxx, minn
    from concourse.dve_uop import DveOpSpec

    for op in dve_ops.OPS:
        if op.name == _OP_NAME:
            return op

    m1 = Src1 * C0
    m2 = Src1 * C1
    spec = Spec(
        body=minn(maxx(Src0 + m1, m1 - m2), m1 + m2) - m1,
        reference=_bfp_q5_reference,
    )
    row = dve_ops._CUSTOM_DVE_ROW_BASE + len(dve_ops.OPS)
    shas = {
        ver: DveOpSpec(
            name=_OP_NAME, opcode=row, uops=lower(spec, ver=ver), rd1_en=True
        ).sha(ver)
        for ver in ("v3", "v4")
    }
    op = DveOp(_OP_NAME, spec, subdim=False, uops_sha=shas)
    dve_ops.OPS.append(op)
    dve_ops.CUSTOM_DVE_SPECS[_OP_NAME] = spec
    dve_ops._SUB_OPCODE_FOR_NAME[_OP_NAME] = row
    return op




# ---------------------------------------------------------------------------
# Tile kernel (per core): x [4, 256, 3136] f32 -> y [4, 256, 3136] bf16
# ---------------------------------------------------------------------------
B_PER_CORE = 4
C_CH = 256
HW = 3136          # 56*56 = N_BIG*PX_BIG + 64
PX_BIG = 512
N_BIG = 6
PX_REM = HW - N_BIG * PX_BIG   # 64


def bfp_tile_kernel(ctx, tc, y_ap, x_ap):
    nc = tc.nc
    op = _register_custom_op()

    const_pool = ctx.enter_context(tc.tile_pool(name="const", bufs=1))
    x_pool = ctx.enter_context(tc.tile_pool(name="xin", bufs=2))
    o_pool = ctx.enter_context(tc.tile_pool(name="osb", bufs=2))
    xt_pool = ctx.enter_context(tc.tile_pool(name="xt", bufs=3, space="PSUM"))
    on_pool = ctx.enter_context(tc.tile_pool(name="on", bufs=2, space="PSUM"))
    q_pool = ctx.enter_context(tc.tile_pool(name="q", bufs=3))
    m_pool = ctx.enter_context(tc.tile_pool(name="m", bufs=6))

    state = {}
    _idents = {}

    def ensure_idents():
        if _idents:
            return
        ident = const_pool.tile([128, 128], F32, name="ident")
        masks.make_identity(nc, ident[:])
        ident_bf = const_pool.tile([128, 128], BF16, name="ident_bf")
        masks.make_identity(nc, ident_bf[:])
        _idents["f32"] = ident
        _idents["bf16"] = ident_bf

    def emit_fwd(b, px0, npx, x_sb, out_sb):
        """Forward PE transposes of tile (b, px0..px0+npx) into PSUM."""
        if npx >= 128:
            nc2 = npx // 128
            xt = xt_pool.tile([128, nc2 * 256], F32, tag="xt", name=f"xt_{b}_{px0}")
            for c2 in range(nc2):
                for h in range(2):
                    seg = (c2 * 2 + h) * 128
                    nc.tensor.matmul(
                        xt[:, seg:seg + 128],
                        x_sb[:, h, px0 + 128 * c2:px0 + 128 * c2 + 128],
                        _idents["f32"][:, :],
                        is_transpose=True,
                    )
        else:
            xt = xt_pool.tile([64, 256], F32, tag="xt", name=f"xt_{b}_{px0}")
            for h in range(2):
                nc.tensor.matmul(
                    xt[:, h * 128:h * 128 + 128],
                    x_sb[:, h, px0:px0 + npx],
                    _idents["f32"][:, :],
                    is_transpose=True,
                )
        state[(b, px0)] = (xt, npx, out_sb)

    def emit_tail(b, px0):
        """Reduce + quantize + backward transposes + copy-out + store."""
        xt, npx, out_sb = state.pop((b, px0))
        big = npx >= 128
        parts = 128 if big else 64
        fd = xt.shape[1]
        nj = fd // 32
        xt3 = xt[:].rearrange("p (j k) -> p j k", k=32)

        mm = m_pool.tile([parts, nj], F32, tag="m" if big else "ms",
                         name=f"mm_{b}_{px0}")
        # split per PSUM bank (512 f32 cols): two bank-local reduces are
        # faster than one 3D-AP reduce spanning banks, and each piece starts
        # as soon as its half of the forward transposes lands.  The first
        # tile splits finer so the DVE starts after only 2 transposes.
        step = 256 if (b, px0) == (0, 0) else 512
        for lo in range(0, fd, step):
            hi = min(lo + step, fd)
            nc.vector.tensor_reduce(
                out=mm[:, lo // 32:hi // 32],
                in_=xt[:, lo:hi].rearrange("p (j k) -> p j k", k=32),
                axis=mybir.AxisListType.X,
                op=mybir.AluOpType.max, apply_absolute_value=True,
            )
        s0 = m_pool.tile([parts, nj], F32, tag="s0" if big else "s0s",
                         name=f"s0_{b}_{px0}")
        nc.vector.tensor_scalar(
            out=s0[:].bitcast(I32), in0=mm[:].bitcast(I32),
            scalar1=0x7F800000, scalar2=None,
            op0=mybir.AluOpType.bitwise_and,
        )
        q = q_pool.tile([parts, nj * 32], BF16, tag="q", name=f"q_{b}_{px0}")
        nc.vector._custom_dve(
            op,
            out=q[:].rearrange("p (j k) -> p j k", k=32),
            in0=xt3,
            in1=s0[:].unsqueeze(-1).broadcast_to([parts, nj, 32]),
            s0=786432.0, s1=1.9375,
        )

        if big:
            nc2 = npx // 128
            on = on_pool.tile([128, fd], BF16, tag="on", name=f"on_{b}_{px0}")
            for c2 in range(nc2):
                for h in range(2):
                    seg = (c2 * 2 + h) * 128
                    nc.tensor.matmul(
                        on[:, seg:seg + 128],
                        q[:, 256 * c2 + 128 * h:256 * c2 + 128 * h + 128],
                        _idents["bf16"][:, :],
                        is_transpose=True,
                    )
            dst = out_sb[:, :, px0:px0 + npx].rearrange(
                "p h (c k) -> p c h k", k=128)
            nc.scalar.activation(dst, on[:], mybir.ActivationFunctionType.Copy)
        else:
            on = on_pool.tile([128, 128], BF16, tag="on", name=f"on_{b}_{px0}")
            for h in range(2):
                nc.tensor.matmul(
                    on[:, h * npx:(h + 1) * npx],
                    q[:, h * 128:h * 128 + 128],
                    _idents["bf16"][:64, :64],
                    is_transpose=True,
                )
            nc.scalar.activation(
                out_sb[:, :, px0:px0 + npx], on[:],
                mybir.ActivationFunctionType.Copy,
            )
        nc.scalar.dma_start(
            out=y_ap[b].rearrange("(h p) w -> p h w", p=128)[
                :, :, px0:px0 + npx],
            in_=out_sb[:, :, px0:px0 + npx],
        )

    # Software-pipelined emission: fwd transposes run two tiles ahead of
    # each tile's tail so the in-order PE queue interleaves them, and input
    # chunks are DMA'd per tile with a two-tile lead.
    full = [PX_BIG] * N_BIG + [PX_REM]
    jobs = []
    for b in range(B_PER_CORE):
        px0 = 0
        for npx in full:
            jobs.append((b, px0, npx))
            px0 += npx
    x_sbs, out_sbs = {}, {}

    def emit_in_chunk(b, px0, npx, split=1):
        if px0 == 0:
            x_sbs[b] = x_pool.tile([128, 2, HW], F32, tag="x", name=f"x_sb{b}")
            out_sbs[b] = o_pool.tile([128, 2, HW], BF16, tag="o", name=f"out_sb{b}")
        xr = x_ap[b].rearrange("(h p) w -> p h w", p=128)
        step = npx // split
        for lo in range(px0, px0 + npx, step):
            nc.sync.dma_start(out=x_sbs[b][:, :, lo:lo + step],
                              in_=xr[:, :, lo:lo + step])

    prefetch = 0
    LAG = 2
    ensure_idents()
    n = len(jobs)
    for i, (b, px0, npx) in enumerate(jobs):
        while prefetch < len(jobs) and prefetch <= i + 2:
            # first tile's load split 4-way: its first 128px chunk (and the
            # fwd transposes on it) become ready ~3x sooner
            emit_in_chunk(*jobs[prefetch], split=4 if prefetch == 0 else 1)
            prefetch += 1
        if i == n - 1:
            # drop to LAG=0 for the finale: the remaining backward/copy/store
            # chains overlap the last tile's transposes and quantize
            emit_tail(*jobs[i - 2][:2])
            emit_tail(*jobs[i - 1][:2])
        emit_fwd(b, px0, npx, x_sbs[b], out_sbs[b])
        if i >= LAG and i < n - 1:
            emit_tail(*jobs[i - LAG][:2])
    emit_tail(*jobs[n - 1][:2])


# ---------------------------------------------------------------------------
# Build + run
# ---------------------------------------------------------------------------
_CACHED = {}


def build_bass(n_cores=8):
    from contextlib import ExitStack

    nc = bacc.Bacc(
        "TRN2",
        target_bir_lowering=False,
        debug=False,
        enable_asserts=False,
        num_devices=n_cores,
    )
    x = nc.dram_tensor("activations", [B_PER_CORE, C_CH, HW], F32,
                       kind="ExternalInput").ap()
    y = nc.dram_tensor("out", [B_PER_CORE, C_CH, HW], BF16,
                       kind="ExternalOutput").ap()
    with tile.TileContext(nc) as tc:
        with ExitStack() as ctx:
            bfp_tile_kernel(ctx, tc, y, x)
    nc.compile()
    return nc


def kernel(activations: np.ndarray) -> np.ndarray:
    x = np.ascontiguousarray(np.asarray(activations), dtype=np.float32)
    B, C, H, W = x.shape            # [32, 256, 56, 56]
    n_cores = 8
    bpc = B // n_cores              # 4
    xs = x.reshape(n_cores, bpc, C, H * W)
    in_maps = [{"activations": np.ascontiguousarray(xs[c])} for c in range(n_cores)]

    if "nc" not in _CACHED:
        _CACHED["nc"] = build_bass(n_cores)
    nc = _CACHED["nc"]

    res = run_bass_kernel_spmd(nc, in_maps, core_ids=list(range(n_cores)))
    out = np.stack([np.asarray(res.results[c]["out"]) for c in range(n_cores)])
    return out.reshape(B, C, H, W).astype(np.float32)



# revision 2
# speedup vs baseline: 1.0050x; 1.0050x over previous
"""Trainium2 Bass kernel: BFP (block-floating-point) activation quantization.

Reference semantics (input NCHW [32, 256, 56, 56] f32):
  per (batch, pixel), channels grouped in blocks of 32:
    maxabs = max |x| over the block
    e      = floor(log2(maxabs))          (guard zero blocks)
    s      = 2^(e-4)                      (5-bit mantissa, QMAX = 31)
    out    = clip(round_half_even(x / s), -31, 31) * s    (0 if maxabs == 0)

v2 design — channel-major, zero data transposes:
  The shared exponent e = max_i floor(log2|x_i|) is recovered through a
  tensor-engine sum instead of a vector-engine max:
      t_i = 2^(8*e_i + 1)   (exact, via two int32 tensor_scalar passes)
      Sigma = sum over the 32-channel block (PE matmul with a 0/1 mask)
      e = floor((log2(Sigma) - 1) / 8)    (exact: carries from summing 32
          terms can move log2 by at most +5 < 8, so the floor-div by 8
          recovers max_i e_i exactly)
  s0 = 2^e is then broadcast back to all 128 partitions with a second
  0/1-mask matmul (f32r, 1 cyc/col), and a single fused DVE custom op does
  the whole round+clip+rescale in one 1x pass over the f32 data:
      C  = s0 * 786432.0        (= 1.5*2^23 * 2^-4 * s0 = magic * s)
      m  = s0 * 1.9375          (= 31 * s)
      out = min(max(x + C, C - m), C + m) - C
  All steps are exact in fp32; outputs are +-q * 2^(e-4) with q <= 31, so
  they are exactly representable in bf16 — the kernel stores bf16 to HBM
  (half the store traffic) and the host widens back to f32.

Per (batch, half) slab [128 ch, 3136 px]:
  passA (DVE 2x):  u = (bits(x) & 0x7F800000) - (111 << 23)   [= (E-111)<<23]
  passB (DVE 2x):  t = (max(u, 0) << 3)                       [= 2^(8e+1)]
  sum   (PE):      8 accumulating matmuls, mask W1_g packs pixel-chunk g's
                   4 block-sums into rows 4g..4g+3 of one [32, 392] PSUM tile
  copy  (ACT):     Sigma PSUM -> SBUF
  extr  (DVE):     e1 = (bits >> 23) + 888 ; s0 = (e1 >> 3) << 23  [= 2^e]
  bcast (PE):      s128[p, j] = s0[4g + p//32, j]  (mask W2_g, f32r)
  quant (DVE 1x):  fused custom op, in0 = x (SBUF f32), in1 = s128 (PSUM)
  store (DMA):     bf16 channel-major, contiguous rows

Sharding: batch 32 -> 4 per core across 8 NeuronCores; no cross-core comms.
"""

import numpy as np

import concourse.bass as bass
import concourse.mybir as mybir
from concourse import bacc, tile
from concourse.bass_utils import run_bass_kernel_spmd

F32 = mybir.dt.float32
F32R = mybir.dt.float32r
BF16 = mybir.dt.bfloat16
I32 = mybir.dt.int32
I16 = mybir.dt.int16

ALU = mybir.AluOpType

# ---------------------------------------------------------------------------
# Custom DVE op: the entire quantize in one 1x pass.
#   out = min(max(Src0 + Src1*C0, Src1*C0 - Src1*C1), Src1*C0 + Src1*C1) - Src1*C0
#   Src0 = x, Src1 = s0 = 2^e, C0 = 786432.0, C1 = 1.9375.
# ---------------------------------------------------------------------------
_OP_NAME = "BFP_Q5_ANT"


def _bfp_q5_reference(in0, in1, s0, s1, imm2):
    in0 = np.asarray(in0, np.float32)
    in1 = np.asarray(in1, np.float32).reshape(in0.shape)
    c = (in1 * np.float32(s0)).astype(np.float32)
    m = (in1 * np.float32(s1)).astype(np.float32)
    u = (in0 + c).astype(np.float32)
    v = np.minimum(np.maximum(u, (c - m).astype(np.float32)),
                   (c + m).astype(np.float32)).astype(np.float32)
    return (v - c).astype(np.float32)


def _register_custom_op():
    import concourse.dve_ops as dve_ops
    from concourse.dve_ops import DveOp
    from concourse.dve_spec import C0, C1, Spec, Src0, Src1, lower, maxx, minn
    from concourse.dve_uop import DveOpSpec

    for op in dve_ops.OPS:
        if op.name == _OP_NAME:
            return op

    m1 = Src1 * C0
    m2 = Src1 * C1
    spec = Spec(
        body=minn(maxx(Src0 + m1, m1 - m2), m1 + m2) - m1,
        reference=_bfp_q5_reference,
    )
    row = dve_ops._CUSTOM_DVE_ROW_BASE + len(dve_ops.OPS)
    shas = {
        ver: DveOpSpec(
            name=_OP_NAME, opcode=row, uops=lower(spec, ver=ver), rd1_en=True
        ).sha(ver)
        for ver in ("v3", "v4")
    }
    op = DveOp(_OP_NAME, spec, subdim=False, uops_sha=shas)
    dve_ops.OPS.append(op)
    dve_ops.CUSTOM_DVE_SPECS[_OP_NAME] = spec
    dve_ops._SUB_OPCODE_FOR_NAME[_OP_NAME] = row
    return op


# ---------------------------------------------------------------------------
# Shapes / constants
# ---------------------------------------------------------------------------
B_PER_CORE = 4
C_CH = 256
HW = 3136              # 56*56
NCH = 392              # pixel chunk for sum/broadcast: 8 per half
NGRP = HW // NCH       # 8
QCH = 784              # pixel chunk for the quantize: 4 per half
NQ = HW // QCH         # 4

EXP_BIAS_SHIFTED = 111 << 23   # (E - 111) << 23 centering; E<=111 <=> e<=-16


def make_aux_inputs():
    """0/1 mask matrices for the block-sum and broadcast matmuls."""
    # W1_g [128, 32]: sums t over each 32-partition block, placing pixel-chunk
    # g's 4 block-sums in output rows 4g..4g+3.
    import ml_dtypes
    w1 = np.zeros((NGRP, 128, 32), np.float32)
    for g in range(NGRP):
        for k in range(128):
            w1[g, k, 4 * g + k // 32] = 1.0
    # W2_g [32, 128]: s128[p] = s0[4g + p//32]
    w2 = np.zeros((NGRP, 32, 128), np.float32)
    for g in range(NGRP):
        for p in range(128):
            w2[g, 4 * g + p // 32, p] = 2.0 ** 111
    return {"w1": w1.astype(ml_dtypes.bfloat16),
            "w2": w2.astype(ml_dtypes.bfloat16)}


# ---------------------------------------------------------------------------
# Tile kernel (per core): x [4, 256, 3136] f32 -> y [4, 256, 3136] bf16
# ---------------------------------------------------------------------------
def bfp_tile_kernel(ctx, tc, y_ap, x_ap, w1_ap, w2_ap):
    nc = tc.nc
    op = _register_custom_op()

    const_pool = ctx.enter_context(tc.tile_pool(name="const", bufs=1))
    x_pool = ctx.enter_context(tc.tile_pool(name="xin", bufs=3))
    o_pool = ctx.enter_context(tc.tile_pool(name="osb", bufs=2))
    t_pool = ctx.enter_context(tc.tile_pool(name="tmp", bufs=2))
    ta_pool = ctx.enter_context(tc.tile_pool(name="tmpa", bufs=2))
    sg_pool = ctx.enter_context(tc.tile_pool(name="sig", bufs=2, space="PSUM"))
    sb_pool = ctx.enter_context(tc.tile_pool(name="sigsb", bufs=2))
    s0_pool = ctx.enter_context(tc.tile_pool(name="s0", bufs=2))
    sp_pool = ctx.enter_context(tc.tile_pool(name="s128", bufs=3, space="PSUM"))

    # Mask weights (constants, loaded once; scalar HWDGE queue so the x
    # loads own the sync queue from instruction 0)
    w1_sb = const_pool.tile([128, NGRP, 32], BF16, name="w1")
    w2_sb = const_pool.tile([32, NGRP, 128], BF16, name="w2")

    def em_wload():
        nc.scalar.dma_start(out=w1_sb[:], in_=w1_ap.rearrange("g p m -> p g m"))
        nc.scalar.dma_start(out=w2_sb[:], in_=w2_ap.rearrange("g k p -> k g p"))

    x_sbs, o_sbs, state = {}, {}, {}
    xb_pool = ctx.enter_context(tc.tile_pool(name="xb", bufs=3))

    def em_load(b, split=1):
        x_sbs[b] = x_pool.tile([128, 2, HW], F32, tag="x", name=f"x{b}")
        o_sbs[b] = o_pool.tile([128, 2, HW], BF16, tag="o", name=f"o{b}")
        xr = x_ap[b].rearrange("(h p) w -> p h w", p=128)
        step = HW // split
        for h in range(2):
            for lo in range(0, HW, step):
                nc.sync.dma_start(out=x_sbs[b][:, h, lo:lo + step],
                                  in_=xr[:, h, lo:lo + step])

    def em_conv(b, h, split=1):
        """ACT compaction: xb = bf16-truncated x (exact hi16 copy)."""
        xhi = (x_sbs[b][:, h, :].bitcast(BF16)
               .rearrange("p (w t) -> p w t", t=2)[:, :, 1])
        xb = xb_pool.tile([128, HW], BF16, tag="xb", name=f"xb{b}{h}")
        step = HW // split
        for lo in range(0, HW, step):
            sl = slice(lo, lo + step)
            nc.scalar.activation(xb[:, sl], xhi[:, sl],
                                 mybir.ActivationFunctionType.Copy)
        state[("xb", b, h)] = xb

    def em_tpath(b, h, split=1):
        """t-passes + block-sum + PSUM->SBUF copy for slab (b, h)."""
        xb = state.pop(("xb", b, h))
        ta = ta_pool.tile([128, HW], BF16, tag="ta", name=f"ta{b}{h}")
        tb = t_pool.tile([128, HW], BF16, tag="tb", name=f"tb{b}{h}")
        step = HW // split
        for lo in range(0, HW, step):
            sl = slice(lo, lo + step)
            # P1 (bitwise): E = (bits16 & 0x7F80) >> 7    (4x: compact bf16)
            nc.vector.tensor_scalar(
                out=ta[:, sl].bitcast(I16), in0=xb[:, sl].bitcast(I16),
                scalar1=0x7F80, scalar2=7,
                op0=ALU.bitwise_and, op1=ALU.logical_shift_right,
            )
            # P2 (arith): m = max(E, 111) - 111   (in-place, 4x mode)
            nc.vector.tensor_scalar(
                out=ta[:, sl].bitcast(I16), in0=ta[:, sl].bitcast(I16),
                scalar1=111, scalar2=111,
                op0=ALU.max, op1=ALU.subtract,
            )
            # P3 (bitwise): t = m << 10       (= 2^(8e+1) as bf16 bits)
            nc.vector.tensor_scalar(
                out=tb[:, sl].bitcast(I16), in0=ta[:, sl].bitcast(I16),
                scalar1=10, scalar2=None,
                op0=ALU.logical_shift_left,
            )
        # block sums: 8 accumulating matmuls into one [32, NCH] PSUM tile
        sg = sg_pool.tile([32, 512], F32, tag="sg", name=f"sg{b}{h}")
        for g in range(NGRP):
            nc.tensor.matmul(
                sg[:, :NCH],
                w1_sb[:, g, :],
                tb[:, g * NCH:(g + 1) * NCH],
                start=(g == 0), stop=(g == NGRP - 1),
            )
        sgs = sb_pool.tile([32, NCH], F32, tag="sgs", name=f"sgs{b}{h}")
        nc.scalar.activation(sgs[:], sg[:, :NCH], mybir.ActivationFunctionType.Copy)
        state[("sgs", b, h)] = sgs

    def em_extract(b, h):
        """extraction E1-E3 for slab (b, h): s0 = 2^e in bf16."""
        sgs = state.pop(("sgs", b, h))
        # es = 2^((E'>>3) - 127) as bf16; the missing 2^111 bias is folded
        # into the w2 mask values, so s128 = es * 2^111 = 2^e.
        shi = (sgs[:].bitcast(I16)
               .rearrange("p (w t) -> p w t", t=2)[:, :, 1])
        s0 = s0_pool.tile([32, NCH], BF16, tag="s0", name=f"s0{b}{h}")
        nc.vector.tensor_scalar(
            out=s0[:].bitcast(I16), in0=shi,
            scalar1=10, scalar2=7,
            op0=ALU.logical_shift_right, op1=ALU.logical_shift_left,
        )
        state[(b, h)] = s0

    def em_bcast(b, h):
        """broadcast matmuls for slab (b, h): s128 chunks into PSUM."""
        s0 = state.pop((b, h))
        sps = []
        for q in range(NQ):
            sp = sp_pool.tile([128, 1024], F32, tag="sp", name=f"sp{b}{h}{q}")
            for i in range(QCH // NCH):
                g = q * (QCH // NCH) + i
                nc.tensor.matmul(
                    sp[:, i * 512:i * 512 + NCH],
                    w2_sb[:, g, :],
                    s0[:],
                )
            sps.append(sp)
        state[("sp", b, h)] = sps

    def em_quant(b, h, chunk_store=False):
        """fused quantize + store for slab (b, h)."""
        sps = state.pop(("sp", b, h))
        yr = y_ap[b].rearrange("(h p) w -> p h w", p=128)
        for q in range(NQ):
            sp3 = sps[q][:].rearrange("p (j k) -> p j k", k=512)[:, :, :NCH]
            qs = slice(q * QCH, (q + 1) * QCH)
            nc.vector._custom_dve(
                op,
                out=o_sbs[b][:, h, qs].rearrange("p (j k) -> p j k", k=NCH),
                in0=x_sbs[b][:, h, qs].rearrange("p (j k) -> p j k", k=NCH),
                in1=sp3,
                s0=786432.0, s1=1.9375,
            )
            if chunk_store:
                nc.sync.dma_start(out=yr[:, h, qs], in_=o_sbs[b][:, h, qs])
        if not chunk_store:
            nc.sync.dma_start(out=yr[:, h, :], in_=o_sbs[b][:, h, :])

    # Software-pipelined emission, one-slab lag between the s-path and the
    # quantize.  Per iteration i the queues see:
    #   DVE: P1-3(i) | E1-3(i-1) | quantize(i-1)
    #   PE : bcast(i-1) [ready immediately] | Sigma(i) [ready after P3(i)]
    #   ACT: copy(i) | store dma(i-1)
    # so the DVE never stalls on the PE->ACT round trip of its own slab.
    slabs = [(b, h) for b in range(B_PER_CORE) for h in range(2)]
    n = len(slabs)
    em_load(0, split=4)
    em_wload()
    em_conv(*slabs[0], split=4)
    for i, (b, h) in enumerate(slabs):
        if h == 0 and b + 1 < B_PER_CORE:
            em_load(b + 1)
        if i >= 2:
            em_bcast(*slabs[i - 2])
        em_tpath(b, h, split=4 if i == 0 else 1)
        if i + 1 < n:
            em_conv(*slabs[i + 1], split=4 if i == 0 else 1)
        if i >= 2:
            em_quant(*slabs[i - 2])
        if i >= 1:
            em_extract(*slabs[i - 1])
    em_bcast(*slabs[-2])
    em_quant(*slabs[-2])
    em_extract(*slabs[-1])
    em_bcast(*slabs[-1])
    em_quant(*slabs[-1], chunk_store=True)


# ---------------------------------------------------------------------------
# Build + run
# ---------------------------------------------------------------------------
_CACHED = {}


def build_bass(n_cores=8):
    from contextlib import ExitStack

    nc = bacc.Bacc(
        "TRN2",
        target_bir_lowering=False,
        debug=False,
        enable_asserts=False,
        num_devices=n_cores,
    )
    x = nc.dram_tensor("activations", [B_PER_CORE, C_CH, HW], F32,
                       kind="ExternalInput").ap()
    w1 = nc.dram_tensor("w1", [NGRP, 128, 32], BF16, kind="ExternalInput").ap()
    w2 = nc.dram_tensor("w2", [NGRP, 32, 128], BF16, kind="ExternalInput").ap()
    y = nc.dram_tensor("out", [B_PER_CORE, C_CH, HW], BF16,
                       kind="ExternalOutput").ap()
    with tile.TileContext(nc) as tc:
        with ExitStack() as ctx:
            bfp_tile_kernel(ctx, tc, y, x, w1, w2)
    nc.compile()
    return nc


def kernel(activations: np.ndarray) -> np.ndarray:
    x = np.ascontiguousarray(np.asarray(activations), dtype=np.float32)
    B, C, H, W = x.shape            # [32, 256, 56, 56]
    n_cores = 8
    bpc = B // n_cores              # 4
    xs = x.reshape(n_cores, bpc, C, H * W)
    aux = make_aux_inputs()
    in_maps = [{"activations": np.ascontiguousarray(xs[c]), **aux}
               for c in range(n_cores)]

    if "nc" not in _CACHED:
        _CACHED["nc"] = build_bass(n_cores)
    nc = _CACHED["nc"]

    res = run_bass_kernel_spmd(nc, in_maps, core_ids=list(range(n_cores)))
    out = np.stack([np.asarray(res.results[c]["out"]) for c in range(n_cores)])
    return out.reshape(B, C, H, W).astype(np.float32)
